# revision 6
# baseline (speedup 1.0000x reference)
"""Trainium2 Bass kernel for nn_CostVolume: H-sharded across 8 NeuronCores.

Algorithm (validated in numpy, 7e-7 vs reference):
- BN folded into conv weights on host.
- down(): 1x1 conv K=1024 matmul -> L (parts 0-63) / Rpad (parts 64-127).
- conv3a collapses: the right half of the cost volume is disparity-shift-
  invariant (conv over d == conv over w on zero-padded R), the left half is
  d-independent away from the mask boundary. Precompute small 2D convs
  G_L/G_R (+first/last d-edge variants), F (left mask-band corrections) and
  E (right W-edge correction); assemble A[d] per-d with DVE adds + ACT relu.
- conv3b: kd=+-1 K-packed via stacked pair tiles S_d=[A[d-1];A[d+1]] (K=128)
  + kd=0 on K=64. fp32r matmuls (full PE rate, ~1e-4 precision).
Each core computes 6 output rows (48 d x 64 ch x 6 h x 160 w).
"""

import sys

sys.path.insert(0, "/opt/trn_rl_repo")

import numpy as np
import concourse.bass as bass
import concourse.bacc as bacc
import concourse.mybir as mybir
from concourse import tile

F32 = mybir.dt.float32
F32R = mybir.dt.float32r
I32 = mybir.dt.int32
RELU = mybir.ActivationFunctionType.Relu
IDENT = mybir.ActivationFunctionType.Identity

H, W, D, CF, CIN = 48, 160, 48, 64, 1024
EPS = 1e-5
NC = 8
HLOC = 6
ROWS_IN = 10  # input rows incl 2-halo each side
ROWS_A = 8  # A rows (out rows +-1)
VLO = -50  # Rpad col range [VLO, 162)
GVLO = -48  # G_R col range [GVLO, 160)
GW = 208
EV0 = 112  # E col range [112, 160)
EW = 48
FW = 52  # F col range [0, 52)
WP = 162  # padded width of S tiles
NEG = -1.0e30

KDSETS = {0: (0, 1, 2), 1: (1, 2), 2: (0, 1)}  # var -> kd indices (mid/first/last)


def _var(d):
    return 1 if d == 0 else (2 if d == D - 1 else 0)


def _fold_bn(w, b, g, beta, m, v):
    s = (g / np.sqrt(v + EPS)).astype(np.float32)
    return (w * s.reshape(-1, *([1] * (w.ndim - 1)))).astype(np.float32), (
        (b - m) * s + beta
    ).astype(np.float32)


def _gr_offsets():
    """[(var, kh, s, weight_builder)] for G_R combined kernels."""
    offs = []
    for var, kds in KDSETS.items():
        ss = sorted({kw - kd for kd in kds for kw in range(3)})
        for kh in range(3):
            for s in ss:
                offs.append((var, kh, s))
    return offs


def _f_combos():
    """[(var, u, [(kh, kw)])] combos with nonzero kernels."""
    combos = []
    for var, kds in KDSETS.items():
        urange = (0, 1) if var == 1 else (-2, -1, 0, 1)
        for u in urange:
            kws = [kw for kw in range(3) if any(kd > u + kw for kd in kds)]
            if kws:
                combos.append((var, u, kws))
    return combos


GR_OFFS = _gr_offsets()
F_COMBOS = _f_combos()
NF = len(F_COMBOS)


def _bcast0(ap, n):
    """Append a step-0 dim of count n to a 2D AP (free-dim broadcast)."""
    return bass.AP(ap.tensor, ap.offset, list(ap.ap) + [[0, n]])


def build_nc():
    nc = bacc.Bacc("TRN2", target_bir_lowering=False, debug=False, num_devices=NC)

    xl_d = nc.dram_tensor("xl", [CIN, ROWS_IN * W], F32, kind="ExternalInput")
    xr_d = nc.dram_tensor("xr", [CIN, ROWS_IN * W], F32, kind="ExternalInput")
    w1t_d = nc.dram_tensor("w1t", [128, 8, CF], F32, kind="ExternalInput")
    wgl_d = nc.dram_tensor("wgl", [CF, 27, CF], F32, kind="ExternalInput")
    wgr_d = nc.dram_tensor("wgr", [CF, len(GR_OFFS), CF], F32, kind="ExternalInput")
    we_d = nc.dram_tensor("we", [CF, 9, CF], F32, kind="ExternalInput")
    wfu_d = nc.dram_tensor("wfu", [CF, NF * 9, CF], F32, kind="ExternalInput")
    wstag_d = nc.dram_tensor("wstag", [128, 9, CF], F32, kind="ExternalInput")
    w0_d = nc.dram_tensor("w0", [CF, 9, CF], F32, kind="ExternalInput")
    b1_d = nc.dram_tensor("b1c", [128, 1], F32, kind="ExternalInput")
    b3a_d = nc.dram_tensor("b3a", [CF, 1], F32, kind="ExternalInput")
    b3b_d = nc.dram_tensor("b3b", [CF, 1], F32, kind="ExternalInput")
    rowm_d = nc.dram_tensor("rowm", [128, ROWS_IN], F32, kind="ExternalInput")
    grm_d = nc.dram_tensor("grm", [128, ROWS_A], F32, kind="ExternalInput")
    y_d = nc.dram_tensor("y", [D, CF, HLOC, W], F32, kind="ExternalOutput")

    with tile.TileContext(nc) as tc:
        with (
            tc.tile_pool(name="wpool", bufs=1) as wpool,
            tc.tile_pool(name="xpool", bufs=4) as xpool,
            tc.tile_pool(name="big", bufs=1) as big,
            tc.tile_pool(name="ost", bufs=4) as ostp,
            tc.tile_pool(name="psd", bufs=2, space="PSUM") as psd_p,
            tc.tile_pool(name="psg", bufs=3, space="PSUM") as psg_p,
        ):
            # ---- weight/aux tiles ----
            w1t = wpool.tile([128, 8, CF], F32R)
            nc.sync.dma_start(w1t[:], w1t_d[:].bitcast(F32R))
            wgl = wpool.tile([CF, 27, CF], F32R)
            nc.sync.dma_start(wgl[:], wgl_d[:].bitcast(F32R))
            wgr = wpool.tile([CF, len(GR_OFFS), CF], F32R)
            nc.sync.dma_start(wgr[:], wgr_d[:].bitcast(F32R))
            wE = wpool.tile([CF, 9, CF], F32R)
            nc.sync.dma_start(wE[:], we_d[:].bitcast(F32R))
            wfu = wpool.tile([CF, NF * 9, CF], F32R)
            nc.sync.dma_start(wfu[:], wfu_d[:].bitcast(F32R))
            wstag = wpool.tile([128, 9, CF], F32R)
            nc.sync.dma_start(wstag[:], wstag_d[:].bitcast(F32R))
            w0 = wpool.tile([CF, 9, CF], F32R)
            nc.sync.dma_start(w0[:], w0_d[:].bitcast(F32R))
            b1 = wpool.tile([CF, 1], F32)
            nc.sync.dma_start(b1[:], b1_d[0:64, :])
            b3a = wpool.tile([CF, 1], F32)
            nc.sync.dma_start(b3a[:], b3a_d[:])
            b3b = wpool.tile([CF, 1], F32)
            nc.sync.dma_start(b3b[:], b3b_d[:])
            rowm = wpool.tile([128, ROWS_IN], F32R)
            nc.sync.dma_start(rowm[:], rowm_d[:].bitcast(F32R))
            grm = wpool.tile([128, ROWS_A], F32)
            nc.sync.dma_start(grm[:], grm_d[:])

            # ---- big persistent tiles ----
            # L at cols [1,161) of 162; R (Rpad) at cols [50,210) of 212; both base 0
            Lp = big.tile([CF, ROWS_IN, WP], F32R)
            Rp = big.tile([CF, ROWS_IN, 212], F32R)
            Gl = big.tile([CF, 3, ROWS_A, W], F32)
            Gr = big.tile([CF, 3, ROWS_A, GW], F32)
            Et = big.tile([CF, 3, ROWS_A, EW], F32)
            Ft = big.tile([CF, NF, ROWS_A, FW], F32)
            ring = [
                big.tile([128, ROWS_A, WP], F32R, tag=f"S{j}", name=f"S{j}")
                for j in range(5)
            ]
            tts = [
                big.tile([CF, ROWS_A, W], F32, tag=f"t{j}", name=f"t{j}")
                for j in range(2)
            ]

            # ---- phase 1: down() ----
            # per (side, 2-row chunk): stream 8 K-chunks through a small xk tile,
            # accumulate into one psum half, ACT-evac with relu+bias into LR.
            for side, x_d in ((0, xl_d), (1, xr_d)):
                for c5 in range(5):
                    ps = psd_p.tile([CF, 2, W], F32, tag="psd")
                    for k in range(8):
                        xk = xpool.tile([128, 2, W], F32R, tag="xk")
                        nc.sync.dma_start(
                            xk[:],
                            x_d[
                                128 * k : 128 * (k + 1), 320 * c5 : 320 * (c5 + 1)
                            ].rearrange("k (r c) -> k r c", r=2).bitcast(F32R),
                        )
                        nc.tensor.matmul(
                            ps[:], w1t[:, k, :], xk[:], start=(k == 0), stop=(k == 7)
                        )
                    r = 2 * c5
                    dst = (
                        Lp[:, r : r + 2, 1:161]
                        if side == 0
                        else Rp[:, r : r + 2, 50:210]
                    )
                    nc.scalar.activation(dst, ps[:], RELU, bias=b1[:])
            # zero pads + row masking
            nc.vector.memset(Lp[:, :, 0:1].bitcast(I32), 0)
            nc.vector.memset(Lp[:, :, 161:162].bitcast(I32), 0)
            nc.vector.memset(Rp[:, :, 0:50].bitcast(I32), 0)
            nc.vector.memset(Rp[:, :, 210:212].bitcast(I32), 0)
            nc.vector.tensor_mul(
                Lp[:, :, 1:161], Lp[:, :, 1:161], _bcast0(rowm[0:64, :], 160)
            )
            nc.vector.tensor_mul(
                Rp[:, :, 50:210], Rp[:, :, 50:210], _bcast0(rowm[0:64, :], 160)
            )

            # ---- phase 2: G_L / G_R / E / F ----
            for var in range(3):
                for c4 in range(4):
                    r = 2 * c4
                    ps = psg_p.tile([CF, 2, GW], F32, tag="psg")
                    for i9 in range(9):
                        kh, kw = divmod(i9, 3)
                        nc.tensor.matmul(
                            ps[:, :, 0:W],
                            wgl[:, 9 * var + i9, :],
                            Lp[:, r + kh : r + kh + 2, kw : kw + W],
                            start=(i9 == 0),
                            stop=(i9 == 8),
                        )
                    nc.scalar.activation(Gl[:, var, r : r + 2, :], ps[:, :, 0:W], IDENT)
            var_offs = {}
            for idx, (var, kh, s) in enumerate(GR_OFFS):
                var_offs.setdefault(var, []).append((idx, kh, s))
            for var in range(3):
                offs = var_offs[var]
                for c4 in range(4):
                    r = 2 * c4
                    ps = psg_p.tile([CF, 2, GW], F32, tag="psg")
                    for j, (idx, kh, s) in enumerate(offs):
                        nc.tensor.matmul(
                            ps[:],
                            wgr[:, idx, :],
                            Rp[:, r + kh : r + kh + 2, s + 2 : s + 2 + GW],
                            start=(j == 0),
                            stop=(j == len(offs) - 1),
                        )
                    # evac + grmask(-1e30 boundary rows) via DVE add
                    nc.vector.tensor_add(
                        Gr[:, var, r : r + 2, :], ps[:], _bcast0(grm[0:64, r : r + 2], GW)
                    )
            # E: one chunk per var (8 rows x 48)
            ei = 0
            for var, kds in KDSETS.items():
                ps = psg_p.tile([CF, ROWS_A, EW], F32, tag="psg")
                n = len(kds) * 3
                j = 0
                for kd in kds:
                    s = 2 - kd
                    for kh in range(3):
                        nc.tensor.matmul(
                            ps[:],
                            wE[:, 3 * (kd) + kh, :],
                            Rp[:, kh : kh + ROWS_A, EV0 + s - VLO : EV0 + s - VLO + EW],
                            start=(j == 0),
                            stop=(j == n - 1),
                        )
                        j += 1
                nc.scalar.activation(Et[:, var, :, :], ps[:], IDENT)
                ei += 1
            # F combos
            for fi, (var, u, kws) in enumerate(F_COMBOS):
                ps = psg_p.tile([CF, ROWS_A, FW], F32, tag="psg")
                n = 3 * len(kws)
                j = 0
                for kh in range(3):
                    for kw in kws:
                        nc.tensor.matmul(
                            ps[:],
                            wfu[:, 9 * fi + 3 * kh + kw, :],
                            Lp[:, kh : kh + ROWS_A, kw : kw + FW],
                            start=(j == 0),
                            stop=(j == n - 1),
                        )
                        j += 1
                nc.scalar.activation(Ft[:, fi, :, :], ps[:], IDENT)
            FIDX = {(var, u): fi for fi, (var, u, _) in enumerate(F_COMBOS)}

            # ---- phase 3: ring memsets ----
            for j in range(5):
                nc.vector.memset(ring[j][:, :, 0:1].bitcast(I32), 0)
                nc.vector.memset(ring[j][:, :, 161:162].bitcast(I32), 0)
            nc.vector.memset(ring[0][0:64, :, 1:161].bitcast(I32), 0)  # A[-1] = 0

            # ---- phase 4: d-loop ----
            for i in range(D + 1):
                if i < D:
                    var = _var(i)
                    bandlo = max(0, i - 2)
                    tt = tts[i % 2]
                    Snext = ring[(i + 1) % 5]
                    # interior+band: t = G_L + G_R(shifted)
                    nc.vector.tensor_add(
                        tt[:, :, bandlo:W],
                        Gl[:, var, :, bandlo:W],
                        Gr[:, var, :, bandlo - i - GVLO : W - i - GVLO],
                    )
                    # band corrections
                    for u in (-2, -1, 0, 1):
                        w = i + u
                        fi = FIDX.get((var, u))
                        if fi is not None and 0 <= w < W:
                            nc.vector.tensor_sub(
                                tt[:, :, w : w + 1],
                                tt[:, :, w : w + 1],
                                Ft[:, fi, :, w : w + 1],
                            )
                    # right W-edge correction at w=159
                    nc.vector.tensor_sub(
                        tt[:, :, W - 1 : W],
                        tt[:, :, W - 1 : W],
                        Et[:, var, :, W - 1 - i - EV0 : W - i - EV0],
                    )
                    # A[i] -> Snext.top
                    if bandlo > 0:
                        nc.scalar.activation(
                            Snext[0:64, :, 1 : 1 + bandlo],
                            Gr[:, var, :, -i - GVLO : bandlo - i - GVLO],
                            RELU,
                            bias=b3a[:],
                        )
                    nc.scalar.activation(
                        Snext[0:64, :, 1 + bandlo : 161],
                        tt[:, :, bandlo:W],
                        RELU,
                        bias=b3a[:],
                    )
                    if i >= 1:
                        nc.sync.dma_start(
                            ring[(i - 1) % 5][64:128, :, :], Snext[0:64, :, :]
                        )
                if i >= 1:
                    d = i - 1
                    if d == D - 1:
                        nc.vector.memset(
                            ring[d % 5][64:128, :, 1:161].bitcast(I32), 0
                        )  # A[48] = 0
                    Td = ring[d % 5]
                    Tn = ring[(d + 1) % 5]
                    for j0 in (0, 3):
                        ps3 = psg_p.tile([CF, 3, W], F32, tag="ps3")
                        for o9 in range(9):
                            kh, kw = divmod(o9, 3)
                            nc.tensor.matmul(
                                ps3[:],
                                wstag[:, o9, :],
                                Td[:, j0 + kh : j0 + kh + 3, kw : kw + W],
                                start=(o9 == 0),
                                stop=False,
                            )
                        for o9 in range(9):
                            kh, kw = divmod(o9, 3)
                            nc.tensor.matmul(
                                ps3[:],
                                w0[:, o9, :],
                                Tn[0:64, j0 + kh : j0 + kh + 3, kw : kw + W],
                                start=False,
                                stop=(o9 == 8),
                            )
                        ost = ostp.tile([CF, 3, W], F32, tag="ost")
                        nc.scalar.activation(ost[:], ps3[:], RELU, bias=b3b[:])
                        nc.sync.dma_start(y_d[d, :, j0 : j0 + 3, :], ost[:])

    nc.finalize()
    return nc


_NC_CACHE = None


def _get_nc():
    global _NC_CACHE
    if _NC_CACHE is None:
        _NC_CACHE = build_nc()
    return _NC_CACHE


def _prep_weights(inputs):
    w1, b1 = _fold_bn(
        inputs["conv1_w"], inputs["conv1_b"], inputs["bn1_g"], inputs["bn1_b"],
        inputs["bn1_m"], inputs["bn1_v"],
    )
    w3a, b3a = _fold_bn(
        inputs["c3a_w"], inputs["c3a_b"], inputs["bn3a_g"], inputs["bn3a_b"],
        inputs["bn3a_m"], inputs["bn3a_v"],
    )
    w3b, b3b = _fold_bn(
        inputs["c3b_w"], inputs["c3b_b"], inputs["bn3b_g"], inputs["bn3b_b"],
        inputs["bn3b_m"], inputs["bn3b_v"],
    )
    wl, wr = w3a[:, :CF], w3a[:, CF:]

    out = {}
    out["w1t"] = np.ascontiguousarray(
        w1.T.reshape(8, 128, CF).transpose(1, 0, 2)
    ).astype(np.float32)
    wgl = np.zeros((CF, 27, CF), np.float32)
    for var, kds in KDSETS.items():
        k = sum(wl[:, :, kd] for kd in kds)  # [o, ci, 3, 3]
        for kh in range(3):
            for kw in range(3):
                wgl[:, 9 * var + 3 * kh + kw, :] = k[:, :, kh, kw].T
    out["wgl"] = wgl
    wgr = np.zeros((CF, len(GR_OFFS), CF), np.float32)
    for idx, (var, kh, s) in enumerate(GR_OFFS):
        kds = KDSETS[var]
        acc = np.zeros((CF, CF), np.float32)
        for kd in kds:
            kw = s + kd
            if 0 <= kw < 3:
                acc += wr[:, :, kd, kh, kw]
        wgr[:, idx, :] = acc.T
    out["wgr"] = wgr
    we = np.zeros((CF, 9, CF), np.float32)
    for kd in range(3):
        for kh in range(3):
            we[:, 3 * kd + kh, :] = wr[:, :, kd, kh, 2].T
    out["we"] = we
    wfu = np.zeros((CF, NF * 9, CF), np.float32)
    for fi, (var, u, kws) in enumerate(F_COMBOS):
        kds = KDSETS[var]
        for kh in range(3):
            for kw in kws:
                acc = np.zeros((CF, CF), np.float32)
                for kd in kds:
                    if kd > u + kw:
                        acc += wl[:, :, kd, kh, kw]
                wfu[:, 9 * fi + 3 * kh + kw, :] = acc.T
    out["wfu"] = wfu
    wstag = np.zeros((128, 9, CF), np.float32)
    for kh in range(3):
        for kw in range(3):
            wstag[0:64, 3 * kh + kw, :] = w3b[:, :, 0, kh, kw].T
            wstag[64:128, 3 * kh + kw, :] = w3b[:, :, 2, kh, kw].T
    out["wstag"] = wstag
    w0t = np.zeros((CF, 9, CF), np.float32)
    for kh in range(3):
        for kw in range(3):
            w0t[:, 3 * kh + kw, :] = w3b[:, :, 1, kh, kw].T
    out["w0"] = w0t
    out["b1c"] = np.concatenate([b1, b1]).reshape(128, 1)
    out["b3a"] = b3a.reshape(CF, 1)
    out["b3b"] = b3b.reshape(CF, 1)
    return out


def _per_core_inputs(inputs, shared, c):
    r0 = 6 * c
    rows = np.arange(r0 - 2, r0 + 8)
    valid = (rows >= 0) & (rows < H)

    def slc(x):
        out = np.zeros((CIN, ROWS_IN, W), np.float32)
        out[:, valid] = x[0][:, rows[valid]]
        return out.reshape(CIN, ROWS_IN * W)

    m = dict(shared)
    m["xl"] = slc(np.asarray(inputs["left_features"], np.float32))
    m["xr"] = slc(np.asarray(inputs["right_features"], np.float32))
    m["rowm"] = np.broadcast_to(
        valid.astype(np.float32), (128, ROWS_IN)
    ).copy()
    arows = np.arange(r0 - 1, r0 + 7)
    gvals = np.where((arows >= 0) & (arows < H), 0.0, NEG).astype(np.float32)
    m["grm"] = np.broadcast_to(gvals, (128, ROWS_A)).copy()
    return m


def kernel(**inputs):
    from concourse.bass_utils import run_bass_kernel_spmd

    nc = _get_nc()
    shared = _prep_weights(inputs)
    in_maps = [_per_core_inputs(inputs, shared, c) for c in range(NC)]
    res = run_bass_kernel_spmd(nc, in_maps, list(range(NC)))
    full = np.zeros((CF, D, H, W), np.float32)
    for c in range(NC):
        y = res.results[c]["y"]  # [48, 64, 6, 160]
        full[:, :, 6 * c : 6 * c + 6, :] = y.transpose(1, 0, 2, 3)
    return full.reshape(1, CF * D, H, W)


# revision 7
# speedup vs baseline: 1.0068x; 1.0068x over previous
"""Trainium2 Bass kernel for nn_CostVolume: H-sharded across 8 NeuronCores.

Algorithm (validated in numpy, 7e-7 vs reference):
- BN folded into conv weights on host.
- down(): 1x1 conv K=1024 matmul -> L (parts 0-63) / Rpad (parts 64-127).
- conv3a collapses: the right half of the cost volume is disparity-shift-
  invariant (conv over d == conv over w on zero-padded R), the left half is
  d-independent away from the mask boundary. Precompute small 2D convs
  G_L/G_R (+first/last d-edge variants), F (left mask-band corrections) and
  E (right W-edge correction); assemble A[d] per-d with DVE adds + ACT relu.
- conv3b: kd=+-1 K-packed via stacked pair tiles S_d=[A[d-1];A[d+1]] (K=128)
  + kd=0 on K=64. fp32r matmuls (full PE rate, ~1e-4 precision).
Each core computes 6 output rows (48 d x 64 ch x 6 h x 160 w).
"""

import sys

sys.path.insert(0, "/opt/trn_rl_repo")

import numpy as np
import concourse.bass as bass
import concourse.bacc as bacc
import concourse.mybir as mybir
from concourse import tile

F32 = mybir.dt.float32
F32R = mybir.dt.float32r
I32 = mybir.dt.int32
RELU = mybir.ActivationFunctionType.Relu
IDENT = mybir.ActivationFunctionType.Identity

H, W, D, CF, CIN = 48, 160, 48, 64, 1024
EPS = 1e-5
NC = 8
HLOC = 6
ROWS_IN = 10  # input rows incl 2-halo each side
ROWS_A = 8  # A rows (out rows +-1)
VLO = -50  # Rpad col range [VLO, 162)
GVLO = -48  # G_R col range [GVLO, 160)
GW = 208
EV0 = 112  # E col range [112, 160)
EW = 48
FW = 52  # F col range [0, 52)
WP = 162  # padded width of S tiles
NEG = -1.0e30

KDSETS = {0: (0, 1, 2), 1: (1, 2), 2: (0, 1)}  # var -> kd indices (mid/first/last)


def _var(d):
    return 1 if d == 0 else (2 if d == D - 1 else 0)


def _fold_bn(w, b, g, beta, m, v):
    s = (g / np.sqrt(v + EPS)).astype(np.float32)
    return (w * s.reshape(-1, *([1] * (w.ndim - 1)))).astype(np.float32), (
        (b - m) * s + beta
    ).astype(np.float32)


def _gr_offsets():
    """[(var, kh, s, weight_builder)] for G_R combined kernels."""
    offs = []
    for var, kds in KDSETS.items():
        ss = sorted({kw - kd for kd in kds for kw in range(3)})
        for kh in range(3):
            for s in ss:
                offs.append((var, kh, s))
    return offs


def _f_combos():
    """[(var, u, [(kh, kw)])] combos with nonzero kernels."""
    combos = []
    for var, kds in KDSETS.items():
        urange = (0, 1) if var == 1 else (-2, -1, 0, 1)
        for u in urange:
            kws = [kw for kw in range(3) if any(kd > u + kw for kd in kds)]
            if kws:
                combos.append((var, u, kws))
    return combos


GR_OFFS = _gr_offsets()
F_COMBOS = _f_combos()
NF = len(F_COMBOS)


def _bcast0(ap, n):
    """Append a step-0 dim of count n to a 2D AP (free-dim broadcast)."""
    return bass.AP(ap.tensor, ap.offset, list(ap.ap) + [[0, n]])


def build_nc():
    nc = bacc.Bacc("TRN2", target_bir_lowering=False, debug=False, num_devices=NC)

    xl_d = nc.dram_tensor("xl", [CIN, ROWS_IN * W], F32, kind="ExternalInput")
    xr_d = nc.dram_tensor("xr", [CIN, ROWS_IN * W], F32, kind="ExternalInput")
    w1t_d = nc.dram_tensor("w1t", [128, 8, CF], F32, kind="ExternalInput")
    wgl_d = nc.dram_tensor("wgl", [CF, 27, CF], F32, kind="ExternalInput")
    wgr_d = nc.dram_tensor("wgr", [CF, len(GR_OFFS), CF], F32, kind="ExternalInput")
    we_d = nc.dram_tensor("we", [CF, 9, CF], F32, kind="ExternalInput")
    wfu_d = nc.dram_tensor("wfu", [CF, NF * 9, CF], F32, kind="ExternalInput")
    wstag_d = nc.dram_tensor("wstag", [128, 9, CF], F32, kind="ExternalInput")
    w0p_d = nc.dram_tensor("w0p", [128, 3, CF], F32, kind="ExternalInput")
    w0m_d = nc.dram_tensor("w0m", [CF, 3, CF], F32, kind="ExternalInput")
    b1_d = nc.dram_tensor("b1c", [128, 1], F32, kind="ExternalInput")
    b3a_d = nc.dram_tensor("b3a", [CF, 1], F32, kind="ExternalInput")
    b3b_d = nc.dram_tensor("b3b", [CF, 1], F32, kind="ExternalInput")
    rowm_d = nc.dram_tensor("rowm", [128, ROWS_IN], F32, kind="ExternalInput")
    grm_d = nc.dram_tensor("grm", [128, ROWS_A], F32, kind="ExternalInput")
    y_d = nc.dram_tensor("y", [D, CF, HLOC, W], F32, kind="ExternalOutput")

    with tile.TileContext(nc) as tc:
        with (
            tc.tile_pool(name="wpool", bufs=1) as wpool,
            tc.tile_pool(name="xpool", bufs=4) as xpool,
            tc.tile_pool(name="big", bufs=1) as big,
            tc.tile_pool(name="ost", bufs=4) as ostp,
            tc.tile_pool(name="psd", bufs=2, space="PSUM") as psd_p,
            tc.tile_pool(name="psg", bufs=3, space="PSUM") as psg_p,
        ):
            # ---- weight/aux tiles ----
            w1t = wpool.tile([128, 8, CF], F32R)
            nc.sync.dma_start(w1t[:], w1t_d[:].bitcast(F32R))
            wgl = wpool.tile([CF, 27, CF], F32R)
            nc.sync.dma_start(wgl[:], wgl_d[:].bitcast(F32R))
            wgr = wpool.tile([CF, len(GR_OFFS), CF], F32R)
            nc.sync.dma_start(wgr[:], wgr_d[:].bitcast(F32R))
            wE = wpool.tile([CF, 9, CF], F32R)
            nc.sync.dma_start(wE[:], we_d[:].bitcast(F32R))
            wfu = wpool.tile([CF, NF * 9, CF], F32R)
            nc.sync.dma_start(wfu[:], wfu_d[:].bitcast(F32R))
            wstag = wpool.tile([128, 9, CF], F32R)
            nc.sync.dma_start(wstag[:], wstag_d[:].bitcast(F32R))
            w0p = wpool.tile([128, 3, CF], F32R)
            nc.sync.dma_start(w0p[:], w0p_d[:].bitcast(F32R))
            w0m = wpool.tile([CF, 3, CF], F32R)
            nc.sync.dma_start(w0m[:], w0m_d[:].bitcast(F32R))
            b1 = wpool.tile([CF, 1], F32)
            nc.sync.dma_start(b1[:], b1_d[0:64, :])
            b3a = wpool.tile([CF, 1], F32)
            nc.sync.dma_start(b3a[:], b3a_d[:])
            b3b = wpool.tile([CF, 1], F32)
            nc.sync.dma_start(b3b[:], b3b_d[:])
            rowm = wpool.tile([128, ROWS_IN], F32R)
            nc.sync.dma_start(rowm[:], rowm_d[:].bitcast(F32R))
            grm = wpool.tile([128, ROWS_A], F32)
            nc.sync.dma_start(grm[:], grm_d[:])

            # ---- big persistent tiles ----
            # L at cols [1,161) of 162; R (Rpad) at cols [50,210) of 212; both base 0
            Lp = big.tile([CF, ROWS_IN, WP], F32R)
            Rp = big.tile([CF, ROWS_IN, 212], F32R)
            Gl = big.tile([CF, 3, ROWS_A, W], F32)
            Gr = big.tile([CF, 3, ROWS_A, GW], F32)
            Et = big.tile([CF, 3, ROWS_A, EW], F32)
            Ft = big.tile([CF, NF, ROWS_A, FW], F32)
            ring = [
                big.tile([128, ROWS_A, WP], F32R, tag=f"S{j}", name=f"S{j}")
                for j in range(5)
            ]
            tts = [
                big.tile([CF, ROWS_A, W], F32, tag=f"t{j}", name=f"t{j}")
                for j in range(2)
            ]
            # Q ring: top = A[d], bottom = A[d] shifted down 2 rows (kh 0/2 K-pack)
            qring = [
                big.tile([128, ROWS_A, WP], F32R, tag=f"Q{j}", name=f"Q{j}")
                for j in range(4)
            ]

            # ---- phase 1: down() ----
            # per (side, 2-row chunk): stream 8 K-chunks through a small xk tile,
            # accumulate into one psum half, ACT-evac with relu+bias into LR.
            for side, x_d in ((0, xl_d), (1, xr_d)):
                for c5 in range(5):
                    ps = psd_p.tile([CF, 2, W], F32, tag="psd")
                    for k in range(8):
                        xk = xpool.tile([128, 2, W], F32R, tag="xk")
                        nc.sync.dma_start(
                            xk[:],
                            x_d[
                                128 * k : 128 * (k + 1), 320 * c5 : 320 * (c5 + 1)
                            ].rearrange("k (r c) -> k r c", r=2).bitcast(F32R),
                        )
                        nc.tensor.matmul(
                            ps[:], w1t[:, k, :], xk[:], start=(k == 0), stop=(k == 7)
                        )
                    r = 2 * c5
                    dst = (
                        Lp[:, r : r + 2, 1:161]
                        if side == 0
                        else Rp[:, r : r + 2, 50:210]
                    )
                    nc.scalar.activation(dst, ps[:], RELU, bias=b1[:])
            # zero pads + row masking
            nc.vector.memset(Lp[:, :, 0:1].bitcast(I32), 0)
            nc.vector.memset(Lp[:, :, 161:162].bitcast(I32), 0)
            nc.vector.memset(Rp[:, :, 0:50].bitcast(I32), 0)
            nc.vector.memset(Rp[:, :, 210:212].bitcast(I32), 0)
            nc.vector.tensor_mul(
                Lp[:, :, 1:161], Lp[:, :, 1:161], _bcast0(rowm[0:64, :], 160)
            )
            nc.vector.tensor_mul(
                Rp[:, :, 50:210], Rp[:, :, 50:210], _bcast0(rowm[0:64, :], 160)
            )

            # ---- phase 2: G_L / G_R / E / F ----
            for var in range(3):
                for c4 in range(4):
                    r = 2 * c4
                    ps = psg_p.tile([CF, 2, GW], F32, tag="psg")
                    for i9 in range(9):
                        kh, kw = divmod(i9, 3)
                        nc.tensor.matmul(
                            ps[:, :, 0:W],
                            wgl[:, 9 * var + i9, :],
                            Lp[:, r + kh : r + kh + 2, kw : kw + W],
                            start=(i9 == 0),
                            stop=(i9 == 8),
                        )
                    nc.scalar.activation(Gl[:, var, r : r + 2, :], ps[:, :, 0:W], IDENT)
            var_offs = {}
            for idx, (var, kh, s) in enumerate(GR_OFFS):
                var_offs.setdefault(var, []).append((idx, kh, s))
            for var in range(3):
                offs = var_offs[var]
                for c4 in range(4):
                    r = 2 * c4
                    ps = psg_p.tile([CF, 2, GW], F32, tag="psg")
                    for j, (idx, kh, s) in enumerate(offs):
                        nc.tensor.matmul(
                            ps[:],
                            wgr[:, idx, :],
                            Rp[:, r + kh : r + kh + 2, s + 2 : s + 2 + GW],
                            start=(j == 0),
                            stop=(j == len(offs) - 1),
                        )
                    # evac + grmask(-1e30 boundary rows) via DVE add
                    nc.vector.tensor_add(
                        Gr[:, var, r : r + 2, :], ps[:], _bcast0(grm[0:64, r : r + 2], GW)
                    )
            # E: one chunk per var (8 rows x 48)
            ei = 0
            for var, kds in KDSETS.items():
                ps = psg_p.tile([CF, ROWS_A, EW], F32, tag="psg")
                n = len(kds) * 3
                j = 0
                for kd in kds:
                    s = 2 - kd
                    for kh in range(3):
                        nc.tensor.matmul(
                            ps[:],
                            wE[:, 3 * (kd) + kh, :],
                            Rp[:, kh : kh + ROWS_A, EV0 + s - VLO : EV0 + s - VLO + EW],
                            start=(j == 0),
                            stop=(j == n - 1),
                        )
                        j += 1
                nc.scalar.activation(Et[:, var, :, :], ps[:], IDENT)
                ei += 1
            # F combos
            for fi, (var, u, kws) in enumerate(F_COMBOS):
                ps = psg_p.tile([CF, ROWS_A, FW], F32, tag="psg")
                n = 3 * len(kws)
                j = 0
                for kh in range(3):
                    for kw in kws:
                        nc.tensor.matmul(
                            ps[:],
                            wfu[:, 9 * fi + 3 * kh + kw, :],
                            Lp[:, kh : kh + ROWS_A, kw : kw + FW],
                            start=(j == 0),
                            stop=(j == n - 1),
                        )
                        j += 1
                nc.scalar.activation(Ft[:, fi, :, :], ps[:], IDENT)
            FIDX = {(var, u): fi for fi, (var, u, _) in enumerate(F_COMBOS)}

            # ---- phase 3: ring memsets ----
            for j in range(5):
                nc.vector.memset(ring[j][:, :, 0:1].bitcast(I32), 0)
                nc.vector.memset(ring[j][:, :, 161:162].bitcast(I32), 0)
            for j in range(4):
                nc.vector.memset(qring[j][:, :, 0:1].bitcast(I32), 0)
                nc.vector.memset(qring[j][:, :, 161:162].bitcast(I32), 0)
            nc.vector.memset(ring[0][0:64, :, 1:161].bitcast(I32), 0)  # A[-1] = 0

            # ---- phase 4: d-loop ----
            for i in range(D + 1):
                if i < D:
                    var = _var(i)
                    bandlo = max(0, i - 2)
                    tt = tts[i % 2]
                    Snext = ring[(i + 1) % 5]
                    # interior+band: t = G_L + G_R(shifted)
                    nc.vector.tensor_add(
                        tt[:, :, bandlo:W],
                        Gl[:, var, :, bandlo:W],
                        Gr[:, var, :, bandlo - i - GVLO : W - i - GVLO],
                    )
                    # band corrections
                    for u in (-2, -1, 0, 1):
                        w = i + u
                        fi = FIDX.get((var, u))
                        if fi is not None and 0 <= w < W:
                            nc.vector.tensor_sub(
                                tt[:, :, w : w + 1],
                                tt[:, :, w : w + 1],
                                Ft[:, fi, :, w : w + 1],
                            )
                    # right W-edge correction at w=159
                    nc.vector.tensor_sub(
                        tt[:, :, W - 1 : W],
                        tt[:, :, W - 1 : W],
                        Et[:, var, :, W - 1 - i - EV0 : W - i - EV0],
                    )
                    # A[i] -> Snext.top
                    if bandlo > 0:
                        nc.scalar.activation(
                            Snext[0:64, :, 1 : 1 + bandlo],
                            Gr[:, var, :, -i - GVLO : bandlo - i - GVLO],
                            RELU,
                            bias=b3a[:],
                        )
                    nc.scalar.activation(
                        Snext[0:64, :, 1 + bandlo : 161],
                        tt[:, :, bandlo:W],
                        RELU,
                        bias=b3a[:],
                    )
                    if i >= 1:
                        nc.sync.dma_start(
                            ring[(i - 1) % 5][64:128, :, :], Snext[0:64, :, :]
                        )
                    Qi = qring[i % 4]
                    nc.sync.dma_start(Qi[0:64, :, :], Snext[0:64, :, :])
                    nc.sync.dma_start(
                        Qi[64:128, 0:6, :], Snext[0:64, 2:8, :]
                    )
                if i >= 1:
                    d = i - 1
                    if d == D - 1:
                        nc.vector.memset(
                            ring[d % 5][64:128, :, 1:161].bitcast(I32), 0
                        )  # A[48] = 0
                    Td = ring[d % 5]
                    Tn = ring[(d + 1) % 5]
                    for j0 in (0, 3):
                        ps3 = psg_p.tile([CF, 3, W], F32, tag="ps3")
                        for o9 in range(9):
                            kh, kw = divmod(o9, 3)
                            nc.tensor.matmul(
                                ps3[:],
                                wstag[:, o9, :],
                                Td[:, j0 + kh : j0 + kh + 3, kw : kw + W],
                                start=(o9 == 0),
                                stop=False,
                            )
                        Qd = qring[d % 4]
                        for kw in range(3):
                            nc.tensor.matmul(
                                ps3[:],
                                w0p[:, kw, :],
                                Qd[:, j0 : j0 + 3, kw : kw + W],
                                start=False,
                                stop=False,
                            )
                        for kw in range(3):
                            nc.tensor.matmul(
                                ps3[:],
                                w0m[:, kw, :],
                                Qd[0:64, j0 + 1 : j0 + 4, kw : kw + W],
                                start=False,
                                stop=(kw == 2),
                            )
                        ost = ostp.tile([CF, 3, W], F32, tag="ost")
                        nc.scalar.activation(ost[:], ps3[:], RELU, bias=b3b[:])
                        nc.sync.dma_start(y_d[d, :, j0 : j0 + 3, :], ost[:])

    nc.finalize()
    return nc


_NC_CACHE = None


def _get_nc():
    global _NC_CACHE
    if _NC_CACHE is None:
        _NC_CACHE = build_nc()
    return _NC_CACHE


def _prep_weights(inputs):
    w1, b1 = _fold_bn(
        inputs["conv1_w"], inputs["conv1_b"], inputs["bn1_g"], inputs["bn1_b"],
        inputs["bn1_m"], inputs["bn1_v"],
    )
    w3a, b3a = _fold_bn(
        inputs["c3a_w"], inputs["c3a_b"], inputs["bn3a_g"], inputs["bn3a_b"],
        inputs["bn3a_m"], inputs["bn3a_v"],
    )
    w3b, b3b = _fold_bn(
        inputs["c3b_w"], inputs["c3b_b"], inputs["bn3b_g"], inputs["bn3b_b"],
        inputs["bn3b_m"], inputs["bn3b_v"],
    )
    wl, wr = w3a[:, :CF], w3a[:, CF:]

    out = {}
    out["w1t"] = np.ascontiguousarray(
        w1.T.reshape(8, 128, CF).transpose(1, 0, 2)
    ).astype(np.float32)
    wgl = np.zeros((CF, 27, CF), np.float32)
    for var, kds in KDSETS.items():
        k = sum(wl[:, :, kd] for kd in kds)  # [o, ci, 3, 3]
        for kh in range(3):
            for kw in range(3):
                wgl[:, 9 * var + 3 * kh + kw, :] = k[:, :, kh, kw].T
    out["wgl"] = wgl
    wgr = np.zeros((CF, len(GR_OFFS), CF), np.float32)
    for idx, (var, kh, s) in enumerate(GR_OFFS):
        kds = KDSETS[var]
        acc = np.zeros((CF, CF), np.float32)
        for kd in kds:
            kw = s + kd
            if 0 <= kw < 3:
                acc += wr[:, :, kd, kh, kw]
        wgr[:, idx, :] = acc.T
    out["wgr"] = wgr
    we = np.zeros((CF, 9, CF), np.float32)
    for kd in range(3):
        for kh in range(3):
            we[:, 3 * kd + kh, :] = wr[:, :, kd, kh, 2].T
    out["we"] = we
    wfu = np.zeros((CF, NF * 9, CF), np.float32)
    for fi, (var, u, kws) in enumerate(F_COMBOS):
        kds = KDSETS[var]
        for kh in range(3):
            for kw in kws:
                acc = np.zeros((CF, CF), np.float32)
                for kd in kds:
                    if kd > u + kw:
                        acc += wl[:, :, kd, kh, kw]
                wfu[:, 9 * fi + 3 * kh + kw, :] = acc.T
    out["wfu"] = wfu
    wstag = np.zeros((128, 9, CF), np.float32)
    for kh in range(3):
        for kw in range(3):
            wstag[0:64, 3 * kh + kw, :] = w3b[:, :, 0, kh, kw].T
            wstag[64:128, 3 * kh + kw, :] = w3b[:, :, 2, kh, kw].T
    out["wstag"] = wstag
    w0p = np.zeros((128, 3, CF), np.float32)
    w0m = np.zeros((CF, 3, CF), np.float32)
    for kw in range(3):
        w0p[0:64, kw, :] = w3b[:, :, 1, 0, kw].T
        w0p[64:128, kw, :] = w3b[:, :, 1, 2, kw].T
        w0m[:, kw, :] = w3b[:, :, 1, 1, kw].T
    out["w0p"] = w0p
    out["w0m"] = w0m
    out["b1c"] = np.concatenate([b1, b1]).reshape(128, 1)
    out["b3a"] = b3a.reshape(CF, 1)
    out["b3b"] = b3b.reshape(CF, 1)
    return out


def _per_core_inputs(inputs, shared, c):
    r0 = 6 * c
    rows = np.arange(r0 - 2, r0 + 8)
    valid = (rows >= 0) & (rows < H)

    def slc(x):
        out = np.zeros((CIN, ROWS_IN, W), np.float32)
        out[:, valid] = x[0][:, rows[valid]]
        return out.reshape(CIN, ROWS_IN * W)

    m = dict(shared)
    m["xl"] = slc(np.asarray(inputs["left_features"], np.float32))
    m["xr"] = slc(np.asarray(inputs["right_features"], np.float32))
    m["rowm"] = np.broadcast_to(
        valid.astype(np.float32), (128, ROWS_IN)
    ).copy()
    arows = np.arange(r0 - 1, r0 + 7)
    gvals = np.where((arows >= 0) & (arows < H), 0.0, NEG).astype(np.float32)
    m["grm"] = np.broadcast_to(gvals, (128, ROWS_A)).copy()
    return m


def kernel(**inputs):
    from concourse.bass_utils import run_bass_kernel_spmd

    nc = _get_nc()
    shared = _prep_weights(inputs)
    in_maps = [_per_core_inputs(inputs, shared, c) for c in range(NC)]
    res = run_bass_kernel_spmd(nc, in_maps, list(range(NC)))
    full = np.zeros((CF, D, H, W), np.float32)
    for c in range(NC):
        y = res.results[c]["y"]  # [48, 64, 6, 160]
        full[:, :, 6 * c : 6 * c + 6, :] = y.transpose(1, 0, 2, 3)
    return full.reshape(1, CF * D, H, W)


# revision 8
# speedup vs baseline: 18027.1642x; 17905.1534x over previous
"""Trainium2 Bass kernel for nn_CostVolume: H-sharded across 8 NeuronCores.

Algorithm (validated in numpy, 7e-7 vs reference):
- BN folded into conv weights on host.
- down(): 1x1 conv K=1024 matmul -> L (parts 0-63) / Rpad (parts 64-127).
- conv3a collapses: the right half of the cost volume is disparity-shift-
  invariant (conv over d == conv over w on zero-padded R), the left half is
  d-independent away from the mask boundary. Precompute small 2D convs
  G_L/G_R (+first/last d-edge variants), F (left mask-band corrections) and
  E (right W-edge correction); assemble A[d] per-d with DVE adds + ACT relu.
- conv3b: kd=+-1 K-packed via stacked pair tiles S_d=[A[d-1];A[d+1]] (K=128)
  + kd=0 on K=64. fp32r matmuls (full PE rate, ~1e-4 precision).
Each core computes 6 output rows (48 d x 64 ch x 6 h x 160 w).
"""

import sys

sys.path.insert(0, "/opt/trn_rl_repo")

import numpy as np
import concourse.bass as bass
import concourse.bacc as bacc
import concourse.mybir as mybir
from concourse import tile

F32 = mybir.dt.float32
F32R = mybir.dt.float32r
I32 = mybir.dt.int32
RELU = mybir.ActivationFunctionType.Relu
IDENT = mybir.ActivationFunctionType.Identity

H, W, D, CF, CIN = 48, 160, 48, 64, 1024
EPS = 1e-5
NC = 8
HLOC = 6
ROWS_IN = 10  # input rows incl 2-halo each side
ROWS_A = 8  # A rows (out rows +-1)
VLO = -50  # Rpad col range [VLO, 162)
GVLO = -48  # G_R col range [GVLO, 160)
GW = 208
EV0 = 112  # E col range [112, 160)
EW = 48
FW = 52  # F col range [0, 52)
WP = 162  # padded width of S tiles
NEG = -1.0e30

KDSETS = {0: (0, 1, 2), 1: (1, 2), 2: (0, 1)}  # var -> kd indices (mid/first/last)


def _var(d):
    return 1 if d == 0 else (2 if d == D - 1 else 0)


def _fold_bn(w, b, g, beta, m, v):
    s = (g / np.sqrt(v + EPS)).astype(np.float32)
    return (w * s.reshape(-1, *([1] * (w.ndim - 1)))).astype(np.float32), (
        (b - m) * s + beta
    ).astype(np.float32)


def _gr_offsets():
    """[(var, kh, s, weight_builder)] for G_R combined kernels."""
    offs = []
    for var, kds in KDSETS.items():
        ss = sorted({kw - kd for kd in kds for kw in range(3)})
        for kh in range(3):
            for s in ss:
                offs.append((var, kh, s))
    return offs


def _f_combos():
    """[(var, u, [(kh, kw)])] combos with nonzero kernels."""
    combos = []
    for var, kds in KDSETS.items():
        urange = (0, 1) if var == 1 else (-2, -1, 0, 1)
        for u in urange:
            kws = [kw for kw in range(3) if any(kd > u + kw for kd in kds)]
            if kws:
                combos.append((var, u, kws))
    return combos


GR_OFFS = _gr_offsets()
F_COMBOS = _f_combos()
NF = len(F_COMBOS)


def _bcast0(ap, n):
    """Append a step-0 dim of count n to a 2D AP (free-dim broadcast)."""
    return bass.AP(ap.tensor, ap.offset, list(ap.ap) + [[0, n]])


def build_nc():
    nc = bacc.Bacc("TRN2", target_bir_lowering=False, debug=False, num_devices=NC)

    xl_d = nc.dram_tensor("xl", [CIN, ROWS_IN * W], F32, kind="ExternalInput")
    xr_d = nc.dram_tensor("xr", [CIN, ROWS_IN * W], F32, kind="ExternalInput")
    w1t_d = nc.dram_tensor("w1t", [128, 8, CF], F32, kind="ExternalInput")
    wgl_d = nc.dram_tensor("wgl", [CF, 27, CF], F32, kind="ExternalInput")
    wgr_d = nc.dram_tensor("wgr", [CF, len(GR_OFFS), CF], F32, kind="ExternalInput")
    we_d = nc.dram_tensor("we", [CF, 9, CF], F32, kind="ExternalInput")
    wfu_d = nc.dram_tensor("wfu", [CF, NF * 9, CF], F32, kind="ExternalInput")
    wstag_d = nc.dram_tensor("wstag", [128, 9, CF], F32, kind="ExternalInput")
    w0p_d = nc.dram_tensor("w0p", [128, 3, CF], F32, kind="ExternalInput")
    w0m_d = nc.dram_tensor("w0m", [CF, 3, CF], F32, kind="ExternalInput")
    b1_d = nc.dram_tensor("b1c", [128, 1], F32, kind="ExternalInput")
    b3a_d = nc.dram_tensor("b3a", [CF, 1], F32, kind="ExternalInput")
    b3b_d = nc.dram_tensor("b3b", [CF, 1], F32, kind="ExternalInput")
    rowm_d = nc.dram_tensor("rowm", [128, ROWS_IN], F32, kind="ExternalInput")
    grm_d = nc.dram_tensor("grm", [128, ROWS_A], F32, kind="ExternalInput")
    y_d = nc.dram_tensor("y", [D, CF, HLOC, W], F32, kind="ExternalOutput")

    with tile.TileContext(nc) as tc:
        with (
            tc.tile_pool(name="wpool", bufs=1) as wpool,
            tc.tile_pool(name="xpool", bufs=4) as xpool,
            tc.tile_pool(name="big", bufs=1) as big,
            tc.tile_pool(name="ost", bufs=4) as ostp,
            tc.tile_pool(name="psd", bufs=2, space="PSUM") as psd_p,
            tc.tile_pool(name="psg", bufs=3, space="PSUM") as psg_p,
        ):
            # ---- weight/aux tiles ----
            w1t = wpool.tile([128, 8, CF], F32R)
            nc.sync.dma_start(w1t[:], w1t_d[:].bitcast(F32R))
            wgl = wpool.tile([CF, 27, CF], F32R)
            nc.sync.dma_start(wgl[:], wgl_d[:].bitcast(F32R))
            wgr = wpool.tile([CF, len(GR_OFFS), CF], F32R)
            nc.sync.dma_start(wgr[:], wgr_d[:].bitcast(F32R))
            wE = wpool.tile([CF, 9, CF], F32R)
            nc.sync.dma_start(wE[:], we_d[:].bitcast(F32R))
            wfu = wpool.tile([CF, NF * 9, CF], F32R)
            nc.sync.dma_start(wfu[:], wfu_d[:].bitcast(F32R))
            wstag = wpool.tile([128, 9, CF], F32R)
            nc.sync.dma_start(wstag[:], wstag_d[:].bitcast(F32R))
            w0p = wpool.tile([128, 3, CF], F32R)
            nc.sync.dma_start(w0p[:], w0p_d[:].bitcast(F32R))
            w0m = wpool.tile([CF, 3, CF], F32R)
            nc.sync.dma_start(w0m[:], w0m_d[:].bitcast(F32R))
            b1 = wpool.tile([CF, 1], F32)
            nc.sync.dma_start(b1[:], b1_d[0:64, :])
            b3a = wpool.tile([CF, 1], F32)
            nc.sync.dma_start(b3a[:], b3a_d[:])
            b3b = wpool.tile([CF, 1], F32)
            nc.sync.dma_start(b3b[:], b3b_d[:])
            rowm = wpool.tile([128, ROWS_IN], F32R)
            nc.sync.dma_start(rowm[:], rowm_d[:].bitcast(F32R))
            grm = wpool.tile([128, ROWS_A], F32)
            nc.sync.dma_start(grm[:], grm_d[:])

            # ---- big persistent tiles ----
            # L at cols [1,161) of 162; R (Rpad) at cols [50,210) of 212; both base 0
            Lp = big.tile([CF, ROWS_IN, WP], F32R)
            Rp = big.tile([CF, ROWS_IN, 212], F32R)
            Gl = big.tile([CF, 3, ROWS_A, W], F32)
            Gr = big.tile([CF, 3, ROWS_A, GW], F32)
            Et = big.tile([CF, 3, ROWS_A, EW], F32)
            Ft = big.tile([CF, NF, ROWS_A, FW], F32)
            ring = [
                big.tile([128, ROWS_A, WP], F32R, tag=f"S{j}", name=f"S{j}")
                for j in range(5)
            ]
            tts = [
                big.tile([CF, ROWS_A, W], F32, tag=f"t{j}", name=f"t{j}")
                for j in range(2)
            ]
            # Q ring: top = A[d], bottom = A[d] shifted down 2 rows (kh 0/2 K-pack)
            qring = [
                big.tile([128, ROWS_A, WP], F32R, tag=f"Q{j}", name=f"Q{j}")
                for j in range(4)
            ]

            # ---- phase 1: down() ----
            # per (side, 2-row chunk): stream 8 K-chunks through a small xk tile,
            # accumulate into one psum half, ACT-evac with relu+bias into LR.
            for side, x_d in ((0, xl_d), (1, xr_d)):
                for c5 in range(5):
                    ps = psd_p.tile([CF, 2, W], F32, tag="psd")
                    for k in range(8):
                        xk = xpool.tile([128, 2, W], F32R, tag="xk")
                        nc.sync.dma_start(
                            xk[:],
                            x_d[
                                128 * k : 128 * (k + 1), 320 * c5 : 320 * (c5 + 1)
                            ].rearrange("k (r c) -> k r c", r=2).bitcast(F32R),
                        )
                        nc.tensor.matmul(
                            ps[:], w1t[:, k, :], xk[:], start=(k == 0), stop=(k == 7)
                        )
                    r = 2 * c5
                    dst = (
                        Lp[:, r : r + 2, 1:161]
                        if side == 0
                        else Rp[:, r : r + 2, 50:210]
                    )
                    nc.scalar.activation(dst, ps[:], RELU, bias=b1[:])
            # zero pads + row masking
            nc.vector.memset(Lp[:, :, 0:1].bitcast(I32), 0)
            nc.vector.memset(Lp[:, :, 161:162].bitcast(I32), 0)
            nc.vector.memset(Rp[:, :, 0:50].bitcast(I32), 0)
            nc.vector.memset(Rp[:, :, 210:212].bitcast(I32), 0)
            nc.vector.tensor_mul(
                Lp[:, :, 1:161], Lp[:, :, 1:161], _bcast0(rowm[0:64, :], 160)
            )
            nc.vector.tensor_mul(
                Rp[:, :, 50:210], Rp[:, :, 50:210], _bcast0(rowm[0:64, :], 160)
            )

            # ---- phase 2: G_L / G_R / E / F ----
            for var in range(3):
                for c4 in range(4):
                    r = 2 * c4
                    ps = psg_p.tile([CF, 2, GW], F32, tag="psg")
                    for i9 in range(9):
                        kh, kw = divmod(i9, 3)
                        nc.tensor.matmul(
                            ps[:, :, 0:W],
                            wgl[:, 9 * var + i9, :],
                            Lp[:, r + kh : r + kh + 2, kw : kw + W],
                            start=(i9 == 0),
                            stop=(i9 == 8),
                        )
                    nc.scalar.activation(Gl[:, var, r : r + 2, :], ps[:, :, 0:W], IDENT)
            var_offs = {}
            for idx, (var, kh, s) in enumerate(GR_OFFS):
                var_offs.setdefault(var, []).append((idx, kh, s))
            for var in range(3):
                offs = var_offs[var]
                for c4 in range(4):
                    r = 2 * c4
                    ps = psg_p.tile([CF, 2, GW], F32, tag="psg")
                    for j, (idx, kh, s) in enumerate(offs):
                        nc.tensor.matmul(
                            ps[:],
                            wgr[:, idx, :],
                            Rp[:, r + kh : r + kh + 2, s + 2 : s + 2 + GW],
                            start=(j == 0),
                            stop=(j == len(offs) - 1),
                        )
                    # evac + grmask(-1e30 boundary rows) via DVE add
                    nc.vector.tensor_add(
                        Gr[:, var, r : r + 2, :], ps[:], _bcast0(grm[0:64, r : r + 2], GW)
                    )
            # E: one chunk per var (8 rows x 48)
            ei = 0
            for var, kds in KDSETS.items():
                ps = psg_p.tile([CF, ROWS_A, EW], F32, tag="psg")
                n = len(kds) * 3
                j = 0
                for kd in kds:
                    s = 2 - kd
                    for kh in range(3):
                        nc.tensor.matmul(
                            ps[:],
                            wE[:, 3 * (kd) + kh, :],
                            Rp[:, kh : kh + ROWS_A, EV0 + s - VLO : EV0 + s - VLO + EW],
                            start=(j == 0),
                            stop=(j == n - 1),
                        )
                        j += 1
                nc.scalar.activation(Et[:, var, :, :], ps[:], IDENT)
                ei += 1
            # F combos
            for fi, (var, u, kws) in enumerate(F_COMBOS):
                ps = psg_p.tile([CF, ROWS_A, FW], F32, tag="psg")
                n = 3 * len(kws)
                j = 0
                for kh in range(3):
                    for kw in kws:
                        nc.tensor.matmul(
                            ps[:],
                            wfu[:, 9 * fi + 3 * kh + kw, :],
                            Lp[:, kh : kh + ROWS_A, kw : kw + FW],
                            start=(j == 0),
                            stop=(j == n - 1),
                        )
                        j += 1
                nc.scalar.activation(Ft[:, fi, :, :], ps[:], IDENT)
            FIDX = {(var, u): fi for fi, (var, u, _) in enumerate(F_COMBOS)}

            # ---- phase 3: ring memsets ----
            for j in range(5):
                nc.vector.memset(ring[j][:, :, 0:1].bitcast(I32), 0)
                nc.vector.memset(ring[j][:, :, 161:162].bitcast(I32), 0)
            for j in range(4):
                nc.vector.memset(qring[j][:, :, 0:1].bitcast(I32), 0)
                nc.vector.memset(qring[j][:, :, 161:162].bitcast(I32), 0)
            nc.vector.memset(ring[0][0:64, :, 1:161].bitcast(I32), 0)  # A[-1] = 0

            # ---- phase 4: d-loop ----
            for i in range(D + 1):
                if i < D:
                    var = _var(i)
                    bandlo = max(0, i - 2)
                    tt = tts[i % 2]
                    Snext = ring[(i + 1) % 5]
                    # interior+band: t = G_L + G_R(shifted)
                    nc.vector.tensor_add(
                        tt[:, :, bandlo:W],
                        Gl[:, var, :, bandlo:W],
                        Gr[:, var, :, bandlo - i - GVLO : W - i - GVLO],
                    )
                    # band corrections
                    for u in (-2, -1, 0, 1):
                        w = i + u
                        fi = FIDX.get((var, u))
                        if fi is not None and 0 <= w < W:
                            nc.vector.tensor_sub(
                                tt[:, :, w : w + 1],
                                tt[:, :, w : w + 1],
                                Ft[:, fi, :, w : w + 1],
                            )
                    # right W-edge correction at w=159
                    nc.vector.tensor_sub(
                        tt[:, :, W - 1 : W],
                        tt[:, :, W - 1 : W],
                        Et[:, var, :, W - 1 - i - EV0 : W - i - EV0],
                    )
                    # A[i] -> Snext.top
                    if bandlo > 0:
                        nc.scalar.activation(
                            Snext[0:64, :, 1 : 1 + bandlo],
                            Gr[:, var, :, -i - GVLO : bandlo - i - GVLO],
                            RELU,
                            bias=b3a[:],
                        )
                    nc.scalar.activation(
                        Snext[0:64, :, 1 + bandlo : 161],
                        tt[:, :, bandlo:W],
                        RELU,
                        bias=b3a[:],
                    )
                    if i >= 1:
                        nc.sync.dma_start(
                            ring[(i - 1) % 5][64:128, :, :], Snext[0:64, :, :]
                        )
                    Qi = qring[i % 4]
                    nc.sync.dma_start(Qi[0:64, :, :], Snext[0:64, :, :])
                    nc.sync.dma_start(
                        Qi[64:128, 0:6, :], Snext[0:64, 2:8, :]
                    )
                if i >= 1:
                    d = i - 1
                    if d == D - 1:
                        nc.vector.memset(
                            ring[d % 5][64:128, :, 1:161].bitcast(I32), 0
                        )  # A[48] = 0
                    Td = ring[d % 5]
                    Tn = ring[(d + 1) % 5]
                    for j0 in (0, 3):
                        ps3 = psg_p.tile([CF, 3, W], F32, tag="ps3")
                        for o9 in range(9):
                            kh, kw = divmod(o9, 3)
                            nc.tensor.matmul(
                                ps3[:],
                                wstag[:, o9, :],
                                Td[:, j0 + kh : j0 + kh + 3, kw : kw + W],
                                start=(o9 == 0),
                                stop=False,
                            )
                        Qd = qring[d % 4]
                        for kw in range(3):
                            nc.tensor.matmul(
                                ps3[:],
                                w0p[:, kw, :],
                                Qd[:, j0 : j0 + 3, kw : kw + W],
                                start=False,
                                stop=False,
                            )
                        for kw in range(3):
                            nc.tensor.matmul(
                                ps3[:],
                                w0m[:, kw, :],
                                Qd[0:64, j0 + 1 : j0 + 4, kw : kw + W],
                                start=False,
                                stop=(kw == 2),
                            )
                        ost = ostp.tile([CF, 3, W], F32, tag="ost")
                        nc.scalar.activation(ost[:], ps3[:], RELU, bias=b3b[:])
                        nc.sync.dma_start(y_d[d, :, j0 : j0 + 3, :], ost[:])

    nc.finalize()
    return nc


_NC_CACHE = None


def _get_nc():
    global _NC_CACHE
    if _NC_CACHE is None:
        _NC_CACHE = build_nc()
    return _NC_CACHE


def _prep_weights(inputs):
    w1, b1 = _fold_bn(
        inputs["conv1_w"], inputs["conv1_b"], inputs["bn1_g"], inputs["bn1_b"],
        inputs["bn1_m"], inputs["bn1_v"],
    )
    w3a, b3a = _fold_bn(
        inputs["c3a_w"], inputs["c3a_b"], inputs["bn3a_g"], inputs["bn3a_b"],
        inputs["bn3a_m"], inputs["bn3a_v"],
    )
    w3b, b3b = _fold_bn(
        inputs["c3b_w"], inputs["c3b_b"], inputs["bn3b_g"], inputs["bn3b_b"],
        inputs["bn3b_m"], inputs["bn3b_v"],
    )
    wl, wr = w3a[:, :CF], w3a[:, CF:]

    out = {}
    out["w1t"] = np.ascontiguousarray(
        w1.T.reshape(8, 128, CF).transpose(1, 0, 2)
    ).astype(np.float32)
    wgl = np.zeros((CF, 27, CF), np.float32)
    for var, kds in KDSETS.items():
        k = sum(wl[:, :, kd] for kd in kds)  # [o, ci, 3, 3]
        for kh in range(3):
            for kw in range(3):
                wgl[:, 9 * var + 3 * kh + kw, :] = k[:, :, kh, kw].T
    out["wgl"] = wgl
    wgr = np.zeros((CF, len(GR_OFFS), CF), np.float32)
    for idx, (var, kh, s) in enumerate(GR_OFFS):
        kds = KDSETS[var]
        acc = np.zeros((CF, CF), np.float32)
        for kd in kds:
            kw = s + kd
            if 0 <= kw < 3:
                acc += wr[:, :, kd, kh, kw]
        wgr[:, idx, :] = acc.T
    out["wgr"] = wgr
    we = np.zeros((CF, 9, CF), np.float32)
    for kd in range(3):
        for kh in range(3):
            we[:, 3 * kd + kh, :] = wr[:, :, kd, kh, 2].T
    out["we"] = we
    wfu = np.zeros((CF, NF * 9, CF), np.float32)
    for fi, (var, u, kws) in enumerate(F_COMBOS):
        kds = KDSETS[var]
        for kh in range(3):
            for kw in kws:
                acc = np.zeros((CF, CF), np.float32)
                for kd in kds:
                    if kd > u + kw:
                        acc += wl[:, :, kd, kh, kw]
                wfu[:, 9 * fi + 3 * kh + kw, :] = acc.T
    out["wfu"] = wfu
    wstag = np.zeros((128, 9, CF), np.float32)
    for kh in range(3):
        for kw in range(3):
            wstag[0:64, 3 * kh + kw, :] = w3b[:, :, 0, kh, kw].T
            wstag[64:128, 3 * kh + kw, :] = w3b[:, :, 2, kh, kw].T
    out["wstag"] = wstag
    w0p = np.zeros((128, 3, CF), np.float32)
    w0m = np.zeros((CF, 3, CF), np.float32)
    for kw in range(3):
        w0p[0:64, kw, :] = w3b[:, :, 1, 0, kw].T
        w0p[64:128, kw, :] = w3b[:, :, 1, 2, kw].T
        w0m[:, kw, :] = w3b[:, :, 1, 1, kw].T
    out["w0p"] = w0p
    out["w0m"] = w0m
    out["b1c"] = np.concatenate([b1, b1]).reshape(128, 1)
    out["b3a"] = b3a.reshape(CF, 1)
    out["b3b"] = b3b.reshape(CF, 1)
    return out


def _per_core_inputs(inputs, shared, c):
    r0 = 6 * c
    rows = np.arange(r0 - 2, r0 + 8)
    valid = (rows >= 0) & (rows < H)

    def slc(x):
        out = np.zeros((CIN, ROWS_IN, W), np.float32)
        out[:, valid] = x[0][:, rows[valid]]
        return out.reshape(CIN, ROWS_IN * W)

    m = dict(shared)
    m["xl"] = slc(np.asarray(inputs["left_features"], np.float32))
    m["xr"] = slc(np.asarray(inputs["right_features"], np.float32))
    m["rowm"] = np.broadcast_to(
        valid.astype(np.float32), (128, ROWS_IN)
    ).copy()
    arows = np.arange(r0 - 1, r0 + 7)
    gvals = np.where((arows >= 0) & (arows < H), 0.0, NEG).astype(np.float32)
    m["grm"] = np.broadcast_to(gvals, (128, ROWS_A)).copy()
    return m


_EXEC_CACHE = None


def _get_exec():
    """Build the SPMD executable once; reuse across kernel() calls."""
    global _EXEC_CACHE
    if _EXEC_CACHE is not None:
        return _EXEC_CACHE
    import jax
    import concourse.mybir as mb
    from concourse import bass2jax
    from jax.experimental.shard_map import shard_map
    from jax.sharding import Mesh, PartitionSpec

    nc = _get_nc()
    bass2jax.install_neuronx_cc_hook()
    partition_name = nc.partition_id_tensor.name if nc.partition_id_tensor else None
    in_names, out_names, out_avals, zero_outs = [], [], [], []
    for alloc in nc.m.functions[0].allocations:
        if not isinstance(alloc, mb.MemoryLocationSet):
            continue
        name = alloc.memorylocations[0].name
        if alloc.kind == "ExternalInput":
            if name != partition_name:
                in_names.append(name)
        elif alloc.kind == "ExternalOutput":
            shape = tuple(alloc.tensor_shape)
            dtype = mb.dt.np(alloc.dtype)
            out_names.append(name)
            out_avals.append(jax.core.ShapedArray(shape, dtype))
            zero_outs.append(np.zeros(shape, dtype))
    n_params = len(in_names)
    all_in = list(in_names) + list(out_names)
    if partition_name is not None:
        all_in.append(partition_name)

    def _body(*args):
        operands = list(args)
        if partition_name is not None:
            operands.append(bass2jax.partition_id_tensor())
        outs = bass2jax._bass_exec_p.bind(
            *operands,
            out_avals=tuple(out_avals),
            in_names=tuple(all_in),
            out_names=tuple(out_names),
            lowering_input_output_aliases=(),
            sim_require_finite=True,
            sim_require_nnan=True,
            nc=nc,
        )
        return tuple(outs)

    devices = jax.devices()[:NC]
    mesh = Mesh(np.asarray(devices), ("core",))
    n_outs = len(out_names)
    sharded = jax.jit(
        shard_map(
            _body,
            mesh=mesh,
            in_specs=(PartitionSpec("core"),) * (n_params + n_outs),
            out_specs=(PartitionSpec("core"),) * n_outs,
            check_rep=False,
        ),
        donate_argnums=tuple(range(n_params, n_params + n_outs)),
        keep_unused=True,
    )
    _EXEC_CACHE = (sharded, in_names, out_names, out_avals, zero_outs)
    return _EXEC_CACHE


def _run(in_maps):
    sharded, in_names, out_names, out_avals, zero_outs = _get_exec()
    concat_in = [
        np.concatenate([np.asarray(in_maps[c][nm]) for c in range(NC)], axis=0)
        for nm in in_names
    ]
    concat_zeros = [
        np.zeros((NC * z.shape[0], *z.shape[1:]), z.dtype) for z in zero_outs
    ]
    out_arrs = sharded(*concat_in, *concat_zeros)
    return [
        {
            nm: np.asarray(out_arrs[i]).reshape(NC, *out_avals[i].shape)[c]
            for i, nm in enumerate(out_names)
        }
        for c in range(NC)
    ]


def kernel(**inputs):
    shared = _prep_weights(inputs)
    in_maps = [_per_core_inputs(inputs, shared, c) for c in range(NC)]
    results = _run(in_maps)
    full = np.zeros((CF, D, H, W), np.float32)
    for c in range(NC):
        y = results[c]["y"]  # [48, 64, 6, 160]
        full[:, :, 6 * c : 6 * c + 6, :] = y.transpose(1, 0, 2, 3)
    return full.reshape(1, CF * D, H, W)


# revision 9
# speedup vs baseline: 18160.1599x; 1.0074x over previous
"""Trainium2 Bass kernel for nn_CostVolume: H-sharded across 8 NeuronCores.

Algorithm (validated in numpy, 7e-7 vs reference):
- BN folded into conv weights on host.
- down(): 1x1 conv K=1024 matmul -> L (parts 0-63) / Rpad (parts 64-127).
- conv3a collapses: the right half of the cost volume is disparity-shift-
  invariant (conv over d == conv over w on zero-padded R), the left half is
  d-independent away from the mask boundary. Precompute small 2D convs
  G_L/G_R (+first/last d-edge variants), F (left mask-band corrections) and
  E (right W-edge correction); assemble A[d] per-d with DVE adds + ACT relu.
- conv3b: kd=+-1 K-packed via stacked pair tiles S_d=[A[d-1];A[d+1]] (K=128)
  + kd=0 on K=64. fp32r matmuls (full PE rate, ~1e-4 precision).
Each core computes 6 output rows (48 d x 64 ch x 6 h x 160 w).
"""

import sys

sys.path.insert(0, "/opt/trn_rl_repo")

import numpy as np
import concourse.bass as bass
import concourse.bacc as bacc
import concourse.mybir as mybir
from concourse import tile

F32 = mybir.dt.float32
F32R = mybir.dt.float32r
I32 = mybir.dt.int32
RELU = mybir.ActivationFunctionType.Relu
IDENT = mybir.ActivationFunctionType.Identity

H, W, D, CF, CIN = 48, 160, 48, 64, 1024
EPS = 1e-5
NC = 8
HLOC = 6
ROWS_IN = 10  # input rows incl 2-halo each side
ROWS_A = 8  # A rows (out rows +-1)
VLO = -50  # Rpad col range [VLO, 162)
GVLO = -48  # G_R col range [GVLO, 160)
GW = 208
EV0 = 112  # E col range [112, 160)
EW = 48
FW = 52  # F col range [0, 52)
WP = 162  # padded width of S tiles
NEG = -1.0e30

KDSETS = {0: (0, 1, 2), 1: (1, 2), 2: (0, 1)}  # var -> kd indices (mid/first/last)


def _var(d):
    return 1 if d == 0 else (2 if d == D - 1 else 0)


def _fold_bn(w, b, g, beta, m, v):
    s = (g / np.sqrt(v + EPS)).astype(np.float32)
    return (w * s.reshape(-1, *([1] * (w.ndim - 1)))).astype(np.float32), (
        (b - m) * s + beta
    ).astype(np.float32)


def _gr_offsets():
    """[(var, kh, s, weight_builder)] for G_R combined kernels."""
    offs = []
    for var, kds in KDSETS.items():
        ss = sorted({kw - kd for kd in kds for kw in range(3)})
        for kh in range(3):
            for s in ss:
                offs.append((var, kh, s))
    return offs


def _f_combos():
    """[(var, u, [(kh, kw)])] combos with nonzero kernels."""
    combos = []
    for var, kds in KDSETS.items():
        urange = (0, 1) if var == 1 else (-2, -1, 0, 1)
        for u in urange:
            kws = [kw for kw in range(3) if any(kd > u + kw for kd in kds)]
            if kws:
                combos.append((var, u, kws))
    return combos


GR_OFFS = _gr_offsets()
F_COMBOS = _f_combos()
NF = len(F_COMBOS)


def _bcast0(ap, n):
    """Append a step-0 dim of count n to a 2D AP (free-dim broadcast)."""
    return bass.AP(ap.tensor, ap.offset, list(ap.ap) + [[0, n]])


def build_nc():
    nc = bacc.Bacc("TRN2", target_bir_lowering=False, debug=False, num_devices=NC)

    xl_d = nc.dram_tensor("xl", [CIN, ROWS_IN * W], F32, kind="ExternalInput")
    xr_d = nc.dram_tensor("xr", [CIN, ROWS_IN * W], F32, kind="ExternalInput")
    w1t_d = nc.dram_tensor("w1t", [128, 8, CF], F32, kind="ExternalInput")
    wgl_d = nc.dram_tensor("wgl", [CF, 27, CF], F32, kind="ExternalInput")
    wgr_d = nc.dram_tensor("wgr", [CF, len(GR_OFFS), CF], F32, kind="ExternalInput")
    we_d = nc.dram_tensor("we", [CF, 9, CF], F32, kind="ExternalInput")
    wfu_d = nc.dram_tensor("wfu", [CF, NF * 9, CF], F32, kind="ExternalInput")
    wstag_d = nc.dram_tensor("wstag", [128, 9, CF], F32, kind="ExternalInput")
    w0p_d = nc.dram_tensor("w0p", [128, 3, CF], F32, kind="ExternalInput")
    w0m_d = nc.dram_tensor("w0m", [CF, 3, CF], F32, kind="ExternalInput")
    b1_d = nc.dram_tensor("b1c", [128, 1], F32, kind="ExternalInput")
    b3a_d = nc.dram_tensor("b3a", [CF, 1], F32, kind="ExternalInput")
    b3b_d = nc.dram_tensor("b3b", [CF, 1], F32, kind="ExternalInput")
    rowm_d = nc.dram_tensor("rowm", [128, ROWS_IN], F32, kind="ExternalInput")
    grm_d = nc.dram_tensor("grm", [128, ROWS_A], F32, kind="ExternalInput")
    y_d = nc.dram_tensor("y", [D, CF, HLOC, W], F32, kind="ExternalOutput")

    with tile.TileContext(nc) as tc:
        with (
            tc.tile_pool(name="wpool", bufs=1) as wpool,
            tc.tile_pool(name="xpool", bufs=8) as xpool,
            tc.tile_pool(name="big", bufs=1) as big,
            tc.tile_pool(name="ost", bufs=4) as ostp,
            tc.tile_pool(name="psd", bufs=1, space="PSUM") as psd_p,
            tc.tile_pool(name="psg", bufs=3, space="PSUM") as psg_p,
        ):
            # ---- weight/aux tiles ----
            w1t = wpool.tile([128, 8, CF], F32R)
            nc.sync.dma_start(w1t[:], w1t_d[:].bitcast(F32R))
            wgl = wpool.tile([CF, 27, CF], F32R)
            nc.sync.dma_start(wgl[:], wgl_d[:].bitcast(F32R))
            wgr = wpool.tile([CF, len(GR_OFFS), CF], F32R)
            nc.sync.dma_start(wgr[:], wgr_d[:].bitcast(F32R))
            wE = wpool.tile([CF, 9, CF], F32R)
            nc.sync.dma_start(wE[:], we_d[:].bitcast(F32R))
            wfu = wpool.tile([CF, NF * 9, CF], F32R)
            nc.sync.dma_start(wfu[:], wfu_d[:].bitcast(F32R))
            wstag = wpool.tile([128, 9, CF], F32R)
            nc.sync.dma_start(wstag[:], wstag_d[:].bitcast(F32R))
            w0p = wpool.tile([128, 3, CF], F32R)
            nc.sync.dma_start(w0p[:], w0p_d[:].bitcast(F32R))
            w0m = wpool.tile([CF, 3, CF], F32R)
            nc.sync.dma_start(w0m[:], w0m_d[:].bitcast(F32R))
            b1 = wpool.tile([CF, 1], F32)
            nc.sync.dma_start(b1[:], b1_d[0:64, :])
            b3a = wpool.tile([CF, 1], F32)
            nc.sync.dma_start(b3a[:], b3a_d[:])
            b3b = wpool.tile([CF, 1], F32)
            nc.sync.dma_start(b3b[:], b3b_d[:])
            rowm = wpool.tile([128, ROWS_IN], F32R)
            nc.sync.dma_start(rowm[:], rowm_d[:].bitcast(F32R))
            grm = wpool.tile([128, ROWS_A], F32)
            nc.sync.dma_start(grm[:], grm_d[:])

            # ---- big persistent tiles ----
            # L at cols [1,161) of 162; R (Rpad) at cols [50,210) of 212; both base 0
            Lp = big.tile([CF, ROWS_IN, WP], F32R)
            Rp = big.tile([CF, ROWS_IN, 212], F32R)
            Gl = big.tile([CF, 3, ROWS_A, W], F32)
            Gr = big.tile([CF, 3, ROWS_A, GW], F32)
            Et = big.tile([CF, 3, ROWS_A, EW], F32)
            Ft = big.tile([CF, NF, ROWS_A, FW], F32)
            ring = [
                big.tile([128, ROWS_A, WP], F32R, tag=f"S{j}", name=f"S{j}")
                for j in range(5)
            ]
            tts = [
                big.tile([CF, ROWS_A, W], F32, tag=f"t{j}", name=f"t{j}")
                for j in range(2)
            ]
            # Q ring: top = A[d], bottom = A[d] shifted down 2 rows (kh 0/2 K-pack)
            qring = [
                big.tile([128, ROWS_A, WP], F32R, tag=f"Q{j}", name=f"Q{j}")
                for j in range(4)
            ]

            # ---- phase 1: down() ----
            # per (side, 2-row chunk): stream 8 K-chunks through a small xk tile,
            # accumulate into one psum half, ACT-evac with relu+bias into LR.
            for side, x_d in ((0, xl_d), (1, xr_d)):
                for c5 in range(5):
                    ps = psd_p.tile([CF, 2, W], F32, tag="psd")
                    for k in range(8):
                        xk = xpool.tile([128, 2, W], F32R, tag="xk")
                        nc.sync.dma_start(
                            xk[:],
                            x_d[
                                128 * k : 128 * (k + 1), 320 * c5 : 320 * (c5 + 1)
                            ].rearrange("k (r c) -> k r c", r=2).bitcast(F32R),
                        )
                        nc.tensor.matmul(
                            ps[:], w1t[:, k, :], xk[:], start=(k == 0), stop=(k == 7)
                        )
                    r = 2 * c5
                    dst = (
                        Lp[:, r : r + 2, 1:161]
                        if side == 0
                        else Rp[:, r : r + 2, 50:210]
                    )
                    nc.scalar.activation(dst, ps[:], RELU, bias=b1[:])
            # zero pads + row masking
            nc.vector.memset(Lp[:, :, 0:1].bitcast(I32), 0)
            nc.vector.memset(Lp[:, :, 161:162].bitcast(I32), 0)
            nc.vector.memset(Rp[:, :, 0:50].bitcast(I32), 0)
            nc.vector.memset(Rp[:, :, 210:212].bitcast(I32), 0)
            nc.vector.tensor_mul(
                Lp[:, :, 1:161], Lp[:, :, 1:161], _bcast0(rowm[0:64, :], 160)
            )
            nc.vector.tensor_mul(
                Rp[:, :, 50:210], Rp[:, :, 50:210], _bcast0(rowm[0:64, :], 160)
            )

            # ---- phase 2: G_L / G_R / E / F ----
            for var in range(3):
                for c4 in range(4):
                    r = 2 * c4
                    ps = psg_p.tile([CF, 2, GW], F32, tag="psg")
                    for i9 in range(9):
                        kh, kw = divmod(i9, 3)
                        nc.tensor.matmul(
                            ps[:, :, 0:W],
                            wgl[:, 9 * var + i9, :],
                            Lp[:, r + kh : r + kh + 2, kw : kw + W],
                            start=(i9 == 0),
                            stop=(i9 == 8),
                        )
                    nc.scalar.activation(Gl[:, var, r : r + 2, :], ps[:, :, 0:W], IDENT)
            var_offs = {}
            for idx, (var, kh, s) in enumerate(GR_OFFS):
                var_offs.setdefault(var, []).append((idx, kh, s))
            for var in range(3):
                offs = var_offs[var]
                for c4 in range(4):
                    r = 2 * c4
                    ps = psg_p.tile([CF, 2, GW], F32, tag="psg")
                    for j, (idx, kh, s) in enumerate(offs):
                        nc.tensor.matmul(
                            ps[:],
                            wgr[:, idx, :],
                            Rp[:, r + kh : r + kh + 2, s + 2 : s + 2 + GW],
                            start=(j == 0),
                            stop=(j == len(offs) - 1),
                        )
                    # evac + grmask(-1e30 boundary rows) via DVE add
                    nc.vector.tensor_add(
                        Gr[:, var, r : r + 2, :], ps[:], _bcast0(grm[0:64, r : r + 2], GW)
                    )
            # E: one chunk per var (8 rows x 48)
            ei = 0
            for var, kds in KDSETS.items():
                ps = psg_p.tile([CF, ROWS_A, EW], F32, tag="psg")
                n = len(kds) * 3
                j = 0
                for kd in kds:
                    s = 2 - kd
                    for kh in range(3):
                        nc.tensor.matmul(
                            ps[:],
                            wE[:, 3 * (kd) + kh, :],
                            Rp[:, kh : kh + ROWS_A, EV0 + s - VLO : EV0 + s - VLO + EW],
                            start=(j == 0),
                            stop=(j == n - 1),
                        )
                        j += 1
                nc.scalar.activation(Et[:, var, :, :], ps[:], IDENT)
                ei += 1
            # F combos
            for fi, (var, u, kws) in enumerate(F_COMBOS):
                ps = psg_p.tile([CF, ROWS_A, FW], F32, tag="psg")
                n = 3 * len(kws)
                j = 0
                for kh in range(3):
                    for kw in kws:
                        nc.tensor.matmul(
                            ps[:],
                            wfu[:, 9 * fi + 3 * kh + kw, :],
                            Lp[:, kh : kh + ROWS_A, kw : kw + FW],
                            start=(j == 0),
                            stop=(j == n - 1),
                        )
                        j += 1
                nc.scalar.activation(Ft[:, fi, :, :], ps[:], IDENT)
            FIDX = {(var, u): fi for fi, (var, u, _) in enumerate(F_COMBOS)}

            # ---- phase 3: ring memsets ----
            for j in range(5):
                nc.vector.memset(ring[j][:, :, 0:1].bitcast(I32), 0)
                nc.vector.memset(ring[j][:, :, 161:162].bitcast(I32), 0)
            for j in range(4):
                nc.vector.memset(qring[j][:, :, 0:1].bitcast(I32), 0)
                nc.vector.memset(qring[j][:, :, 161:162].bitcast(I32), 0)
            nc.vector.memset(ring[0][0:64, :, 1:161].bitcast(I32), 0)  # A[-1] = 0

            # ---- phase 4: d-loop ----
            for i in range(D + 1):
                if i < D:
                    var = _var(i)
                    bandlo = max(0, i - 2)
                    tt = tts[i % 2]
                    Snext = ring[(i + 1) % 5]
                    # interior+band: t = G_L + G_R(shifted)
                    nc.vector.tensor_add(
                        tt[:, :, bandlo:W],
                        Gl[:, var, :, bandlo:W],
                        Gr[:, var, :, bandlo - i - GVLO : W - i - GVLO],
                    )
                    # band corrections
                    for u in (-2, -1, 0, 1):
                        w = i + u
                        fi = FIDX.get((var, u))
                        if fi is not None and 0 <= w < W:
                            nc.vector.tensor_sub(
                                tt[:, :, w : w + 1],
                                tt[:, :, w : w + 1],
                                Ft[:, fi, :, w : w + 1],
                            )
                    # right W-edge correction at w=159
                    nc.vector.tensor_sub(
                        tt[:, :, W - 1 : W],
                        tt[:, :, W - 1 : W],
                        Et[:, var, :, W - 1 - i - EV0 : W - i - EV0],
                    )
                    # A[i] -> Snext.top
                    if bandlo > 0:
                        nc.scalar.activation(
                            Snext[0:64, :, 1 : 1 + bandlo],
                            Gr[:, var, :, -i - GVLO : bandlo - i - GVLO],
                            RELU,
                            bias=b3a[:],
                        )
                    nc.scalar.activation(
                        Snext[0:64, :, 1 + bandlo : 161],
                        tt[:, :, bandlo:W],
                        RELU,
                        bias=b3a[:],
                    )
                    if i >= 1:
                        nc.sync.dma_start(
                            ring[(i - 1) % 5][64:128, :, :], Snext[0:64, :, :]
                        )
                    Qi = qring[i % 4]
                    nc.sync.dma_start(Qi[0:64, :, :], Snext[0:64, :, :])
                    nc.sync.dma_start(
                        Qi[64:128, 0:6, :], Snext[0:64, 2:8, :]
                    )
                if i >= 1:
                    d = i - 1
                    if d == D - 1:
                        nc.vector.memset(
                            ring[d % 5][64:128, :, 1:161].bitcast(I32), 0
                        )  # A[48] = 0
                    Td = ring[d % 5]
                    Tn = ring[(d + 1) % 5]
                    for j0 in (0, 3):
                        ps3 = psg_p.tile([CF, 3, W], F32, tag="ps3", bufs=4)
                        for o9 in range(9):
                            kh, kw = divmod(o9, 3)
                            nc.tensor.matmul(
                                ps3[:],
                                wstag[:, o9, :],
                                Td[:, j0 + kh : j0 + kh + 3, kw : kw + W],
                                start=(o9 == 0),
                                stop=False,
                            )
                        Qd = qring[d % 4]
                        for kw in range(3):
                            nc.tensor.matmul(
                                ps3[:],
                                w0p[:, kw, :],
                                Qd[:, j0 : j0 + 3, kw : kw + W],
                                start=False,
                                stop=False,
                            )
                        for kw in range(3):
                            nc.tensor.matmul(
                                ps3[:],
                                w0m[:, kw, :],
                                Qd[0:64, j0 + 1 : j0 + 4, kw : kw + W],
                                start=False,
                                stop=(kw == 2),
                            )
                        ost = ostp.tile([CF, 3, W], F32, tag="ost")
                        nc.scalar.activation(ost[:], ps3[:], RELU, bias=b3b[:])
                        nc.sync.dma_start(y_d[d, :, j0 : j0 + 3, :], ost[:])

    nc.finalize()
    return nc


_NC_CACHE = None


def _get_nc():
    global _NC_CACHE
    if _NC_CACHE is None:
        _NC_CACHE = build_nc()
    return _NC_CACHE


def _prep_weights(inputs):
    w1, b1 = _fold_bn(
        inputs["conv1_w"], inputs["conv1_b"], inputs["bn1_g"], inputs["bn1_b"],
        inputs["bn1_m"], inputs["bn1_v"],
    )
    w3a, b3a = _fold_bn(
        inputs["c3a_w"], inputs["c3a_b"], inputs["bn3a_g"], inputs["bn3a_b"],
        inputs["bn3a_m"], inputs["bn3a_v"],
    )
    w3b, b3b = _fold_bn(
        inputs["c3b_w"], inputs["c3b_b"], inputs["bn3b_g"], inputs["bn3b_b"],
        inputs["bn3b_m"], inputs["bn3b_v"],
    )
    wl, wr = w3a[:, :CF], w3a[:, CF:]

    out = {}
    out["w1t"] = np.ascontiguousarray(
        w1.T.reshape(8, 128, CF).transpose(1, 0, 2)
    ).astype(np.float32)
    wgl = np.zeros((CF, 27, CF), np.float32)
    for var, kds in KDSETS.items():
        k = sum(wl[:, :, kd] for kd in kds)  # [o, ci, 3, 3]
        for kh in range(3):
            for kw in range(3):
                wgl[:, 9 * var + 3 * kh + kw, :] = k[:, :, kh, kw].T
    out["wgl"] = wgl
    wgr = np.zeros((CF, len(GR_OFFS), CF), np.float32)
    for idx, (var, kh, s) in enumerate(GR_OFFS):
        kds = KDSETS[var]
        acc = np.zeros((CF, CF), np.float32)
        for kd in kds:
            kw = s + kd
            if 0 <= kw < 3:
                acc += wr[:, :, kd, kh, kw]
        wgr[:, idx, :] = acc.T
    out["wgr"] = wgr
    we = np.zeros((CF, 9, CF), np.float32)
    for kd in range(3):
        for kh in range(3):
            we[:, 3 * kd + kh, :] = wr[:, :, kd, kh, 2].T
    out["we"] = we
    wfu = np.zeros((CF, NF * 9, CF), np.float32)
    for fi, (var, u, kws) in enumerate(F_COMBOS):
        kds = KDSETS[var]
        for kh in range(3):
            for kw in kws:
                acc = np.zeros((CF, CF), np.float32)
                for kd in kds:
                    if kd > u + kw:
                        acc += wl[:, :, kd, kh, kw]
                wfu[:, 9 * fi + 3 * kh + kw, :] = acc.T
    out["wfu"] = wfu
    wstag = np.zeros((128, 9, CF), np.float32)
    for kh in range(3):
        for kw in range(3):
            wstag[0:64, 3 * kh + kw, :] = w3b[:, :, 0, kh, kw].T
            wstag[64:128, 3 * kh + kw, :] = w3b[:, :, 2, kh, kw].T
    out["wstag"] = wstag
    w0p = np.zeros((128, 3, CF), np.float32)
    w0m = np.zeros((CF, 3, CF), np.float32)
    for kw in range(3):
        w0p[0:64, kw, :] = w3b[:, :, 1, 0, kw].T
        w0p[64:128, kw, :] = w3b[:, :, 1, 2, kw].T
        w0m[:, kw, :] = w3b[:, :, 1, 1, kw].T
    out["w0p"] = w0p
    out["w0m"] = w0m
    out["b1c"] = np.concatenate([b1, b1]).reshape(128, 1)
    out["b3a"] = b3a.reshape(CF, 1)
    out["b3b"] = b3b.reshape(CF, 1)
    return out


def _per_core_inputs(inputs, shared, c):
    r0 = 6 * c
    rows = np.arange(r0 - 2, r0 + 8)
    valid = (rows >= 0) & (rows < H)

    def slc(x):
        out = np.zeros((CIN, ROWS_IN, W), np.float32)
        out[:, valid] = x[0][:, rows[valid]]
        return out.reshape(CIN, ROWS_IN * W)

    m = dict(shared)
    m["xl"] = slc(np.asarray(inputs["left_features"], np.float32))
    m["xr"] = slc(np.asarray(inputs["right_features"], np.float32))
    m["rowm"] = np.broadcast_to(
        valid.astype(np.float32), (128, ROWS_IN)
    ).copy()
    arows = np.arange(r0 - 1, r0 + 7)
    gvals = np.where((arows >= 0) & (arows < H), 0.0, NEG).astype(np.float32)
    m["grm"] = np.broadcast_to(gvals, (128, ROWS_A)).copy()
    return m


_EXEC_CACHE = None


def _get_exec():
    """Build the SPMD executable once; reuse across kernel() calls."""
    global _EXEC_CACHE
    if _EXEC_CACHE is not None:
        return _EXEC_CACHE
    import jax
    import concourse.mybir as mb
    from concourse import bass2jax
    from jax.experimental.shard_map import shard_map
    from jax.sharding import Mesh, PartitionSpec

    nc = _get_nc()
    bass2jax.install_neuronx_cc_hook()
    partition_name = nc.partition_id_tensor.name if nc.partition_id_tensor else None
    in_names, out_names, out_avals, zero_outs = [], [], [], []
    for alloc in nc.m.functions[0].allocations:
        if not isinstance(alloc, mb.MemoryLocationSet):
            continue
        name = alloc.memorylocations[0].name
        if alloc.kind == "ExternalInput":
            if name != partition_name:
                in_names.append(name)
        elif alloc.kind == "ExternalOutput":
            shape = tuple(alloc.tensor_shape)
            dtype = mb.dt.np(alloc.dtype)
            out_names.append(name)
            out_avals.append(jax.core.ShapedArray(shape, dtype))
            zero_outs.append(np.zeros(shape, dtype))
    n_params = len(in_names)
    all_in = list(in_names) + list(out_names)
    if partition_name is not None:
        all_in.append(partition_name)

    def _body(*args):
        operands = list(args)
        if partition_name is not None:
            operands.append(bass2jax.partition_id_tensor())
        outs = bass2jax._bass_exec_p.bind(
            *operands,
            out_avals=tuple(out_avals),
            in_names=tuple(all_in),
            out_names=tuple(out_names),
            lowering_input_output_aliases=(),
            sim_require_finite=True,
            sim_require_nnan=True,
            nc=nc,
        )
        return tuple(outs)

    devices = jax.devices()[:NC]
    mesh = Mesh(np.asarray(devices), ("core",))
    n_outs = len(out_names)
    sharded = jax.jit(
        shard_map(
            _body,
            mesh=mesh,
            in_specs=(PartitionSpec("core"),) * (n_params + n_outs),
            out_specs=(PartitionSpec("core"),) * n_outs,
            check_rep=False,
        ),
        donate_argnums=tuple(range(n_params, n_params + n_outs)),
        keep_unused=True,
    )
    _EXEC_CACHE = (sharded, in_names, out_names, out_avals, zero_outs)
    return _EXEC_CACHE


def _run(in_maps):
    sharded, in_names, out_names, out_avals, zero_outs = _get_exec()
    concat_in = [
        np.concatenate([np.asarray(in_maps[c][nm]) for c in range(NC)], axis=0)
        for nm in in_names
    ]
    concat_zeros = [
        np.zeros((NC * z.shape[0], *z.shape[1:]), z.dtype) for z in zero_outs
    ]
    out_arrs = sharded(*concat_in, *concat_zeros)
    return [
        {
            nm: np.asarray(out_arrs[i]).reshape(NC, *out_avals[i].shape)[c]
            for i, nm in enumerate(out_names)
        }
        for c in range(NC)
    ]


def kernel(**inputs):
    shared = _prep_weights(inputs)
    in_maps = [_per_core_inputs(inputs, shared, c) for c in range(NC)]
    results = _run(in_maps)
    full = np.zeros((CF, D, H, W), np.float32)
    for c in range(NC):
        y = results[c]["y"]  # [48, 64, 6, 160]
        full[:, :, 6 * c : 6 * c + 6, :] = y.transpose(1, 0, 2, 3)
    return full.reshape(1, CF * D, H, W)
